# revision 15
# baseline (speedup 1.0000x reference)
"""GAT message-passing kernel for 8 Trainium2 NeuronCores (v3).

Strategy (dst-sharded, quad-packed bf16 table, single chunk):
  - Nodes partitioned across 8 cores by destination id (12500 per core).
    Per core, dsts (+44 pads) are sorted by total in-degree; "rank" order
    is used everywhere on device; the host unpermutes rows at the end.
  - Phase 0: each core computes PSUM = x_slot @ Wpack where Wpack packs
    [h0(15) a_src0 h1 a1 h2 a2 h3 a3 | a_dst(4)] so the packed bf16 table
    row needs no reshuffling.  Rows (64 bf16 = 128B) are written in rank
    order; AllGather produces the full 100352-row bf16 table, viewed as
    25088 quad-rows of 256B.
  - Phase 1: CSR slot grid [128 dst x D cols] per tile, degree-sorted, so
    padding is ~3%.  One dma_gather per group fetches QUADS (4 rows/256B
    descriptor, halving nothing but enabling int16 quad ids over the whole
    table -> no chunking, no regather).  A 4-way mask blend (masks streamed
    from host) selects the right row.  Attention weights
    w = exp(leaky(a_src+a_dst)) (softmax normalization cancels), then
    per-tile reduces produce numerator [64] and denominator [4] partials
    accumulated straight into rank order.
  - Phase 2: self-loop contributions and pad corrections are applied
    analytically; out = elu(num/den + bias) @ lin_w + lin_b -> log_softmax.
"""
import sys

sys.path.insert(0, "/opt/trn_rl_repo")

import numpy as np

N, E = 100000, 1600000
IN_DIM, HEADS, HID, OUT_DIM = 128, 4, 15, 10
NEG_SLOPE = 0.2
NCORES = 8
NPC = N // NCORES              # 12500 dst nodes per core
T = (NPC + 127) // 128         # 98 tiles
NPCP = 128 * T                 # 12544 ranks (incl. 44 pads)
SH = NPCP                      # full shard rows contributed to allgather
NTAB = NCORES * SH             # 100352 table rows
NQUAD = NTAB // 4              # 25088 quad rows
ROW = 64                       # packed row: 4 x [h(15) | a_src(1)]
F = HEADS * HID                # 60
GROUP_COLS = 64                # max CSR columns per dma_gather
RUN_COLS = 48


def _wrap_idx(flat):
    """int32 flat index list -> [128, n//16] int16 wrapped layout."""
    n = len(flat)
    assert n % 16 == 0
    w = flat.astype(np.int16).reshape(n // 16, 16).T.copy()
    return np.tile(w, (8, 1))


def _preprocess(src, dst):
    """Rank orders + slot schedule. Returns (D, per_core)."""
    core_of = dst // NPC
    per_core = []
    D = np.zeros(T, np.int64)
    for c in range(NCORES):
        m = core_of == c
        s_c = src[m].astype(np.int64)
        d_loc = (dst[m] - c * NPC).astype(np.int64)
        counts = np.bincount(d_loc, minlength=NPCP).astype(np.int64)
        counts[NPC:] = -1          # pads sort last
        order = np.argsort(-counts, kind="stable").astype(np.int32)
        oi = np.empty(NPCP, np.int32)
        oi[order] = np.arange(NPCP, dtype=np.int32)
        counts = np.maximum(counts, 0)
        D_c = counts[order[0::128]]
        D = np.maximum(D, D_c)
        per_core.append(dict(s=s_c, d=d_loc, counts=counts, order=order,
                             oi=oi))
    return D.astype(np.int32), per_core


def _build_core_arrays(D, info, rank_of_global):
    """Per-core gather quad indices, mask planes, pad counts."""
    base = np.concatenate([[0], np.cumsum(128 * D.astype(np.int64))])
    NI = int(base[-1])
    COLS = NI // 128
    pad_quad = 12500 // 4  # shard 0, rows 12500-12503 (all-zero pad ranks)
    idx_flat = np.full(NI, pad_quad, np.int32)
    sub_flat = np.zeros(NI, np.int8)
    o = np.argsort(info["d"], kind="stable")
    dks = info["d"][o]
    srcs = info["s"][o]
    starts = np.concatenate([[0], np.cumsum(info["counts"])]).astype(np.int64)
    rank = np.arange(len(dks), dtype=np.int64) - starts[dks]
    oi = info["oi"][dks]
    tau, pp = oi // 128, oi % 128
    linear = base[tau] + rank * 128 + pp
    trow = rank_of_global[srcs]            # global table row of each source
    idx_flat[linear] = trow // 4
    sub_flat[linear] = trow % 4
    gidx = _wrap_idx(idx_flat)
    # masks interleaved [128, COLS, 4] int8; [p, col, q] = 1 if quad-sub == q
    masks = np.zeros((128, COLS, 4), np.int8)
    cols = np.arange(NI, dtype=np.int64)
    p_of = cols % 128
    c_of = cols // 128
    masks[p_of, c_of, sub_flat.astype(np.int64)] = 1.0
    # pad counts per (p, tau): pads contribute exp(leaky(a_dst)) to den
    P = (D[None, :].astype(np.int64) - np.zeros((128, T), np.int64))
    cnt_grid = np.zeros((128, T), np.int64)
    np.add.at(cnt_grid, (pp, tau), 1)
    Parr = (D[None, :] - cnt_grid).astype(np.float32)
    return gidx, masks, Parr, COLS


def _groups_of(D):
    groups = []
    cur, cols = [], 0
    for tau in range(T):
        dcol = int(D[tau])
        if dcol == 0:
            continue
        if cols + dcol > GROUP_COLS and cur:
            groups.append(cur)
            cur, cols = [], 0
        cur.append(tau)
        cols += dcol
    if cur:
        groups.append(cur)
    return groups


def _runs_of(D, group):
    runs = []
    i = 0
    while i < len(group):
        Dv = int(D[group[i]])
        j, cols = i, 0
        while j < len(group) and int(D[group[j]]) == Dv \
                and cols + Dv <= RUN_COLS:
            cols += Dv
            j += 1
        if j == i:
            j = i + 1
        runs.append((group[i], j - i, Dv))
        i = j
    return runs


def _build_program(D, COLS):
    import concourse.bass as bass
    import concourse.bacc as bacc
    import concourse.tile as tile
    from concourse import mybir
    from concourse.masks import make_identity

    fp32 = mybir.dt.float32
    bf16 = mybir.dt.bfloat16
    i16 = mybir.dt.int16
    AL = mybir.AluOpType
    AF = mybir.ActivationFunctionType

    NI = 128 * COLS

    nc = bacc.Bacc("TRN2", target_bir_lowering=False, debug=False,
                   num_devices=NCORES)

    xT = nc.dram_tensor("xT", [128, NPCP], fp32, kind="ExternalInput").ap()
    # wpack = [interleaved W|ws (64) | wd (4)]
    w_in = nc.dram_tensor("w_in", [128, ROW + HEADS], fp32,
                          kind="ExternalInput").ap()
    bias_in = nc.dram_tensor("bias_in", [128, ROW], fp32, kind="ExternalInput").ap()
    linw_in = nc.dram_tensor("linw_in", [ROW, OUT_DIM], fp32, kind="ExternalInput").ap()
    linb_in = nc.dram_tensor("linb_in", [128, OUT_DIM], fp32, kind="ExternalInput").ap()
    gidx_in = nc.dram_tensor("gidx_in", [128, NI // 16], i16, kind="ExternalInput").ap()
    mask_in = nc.dram_tensor("mask_in", [128, 4 * COLS], mybir.dt.int8, kind="ExternalInput").ap()
    p_in = nc.dram_tensor("p_in", [128, T], fp32, kind="ExternalInput").ap()
    out_t = nc.dram_tensor("out", [128, T, OUT_DIM], fp32, kind="ExternalOutput").ap()

    tshard = nc.dram_tensor("tshard", [SH, ROW], bf16)
    agout = nc.dram_tensor("agout", [NTAB, ROW], bf16, addr_space="Shared")
    # quad view: [NQUAD, 256] bf16 rows of 512B for the gather
    tableQ = bass.AP(tensor=agout, offset=0,
                     ap=[[4 * ROW, NQUAD], [1, 4 * ROW]])

    with tile.TileContext(nc) as tc:
        from contextlib import ExitStack
        with ExitStack() as ctx:
            singles = ctx.enter_context(tc.tile_pool(name="singles", bufs=1))
            w_sb = singles.tile([128, ROW + HEADS], fp32)
            nc.sync.dma_start(out=w_sb[:], in_=w_in)
            bias_sb = singles.tile([128, ROW], fp32)
            nc.sync.dma_start(out=bias_sb[:], in_=bias_in)
            linw_sb = singles.tile([ROW, OUT_DIM], fp32)
            nc.sync.dma_start(out=linw_sb[:], in_=linw_in)
            linb_sb = singles.tile([128, OUT_DIM], fp32)
            nc.sync.dma_start(out=linb_sb[:], in_=linb_in)
            p_sb = singles.tile([128, T], fp32)
            nc.sync.dma_start(out=p_sb[:], in_=p_in)
            ident = singles.tile([128, 128], fp32)
            make_identity(nc, ident[:])

            tstag = singles.tile([128, T, ROW], bf16)
            adst_nat = singles.tile([128, T, HEADS], fp32)
            # sstag: [num64 | den4]
            sstag = singles.tile([128, T, ROW + HEADS], fp32)
            nc.vector.memset(sstag[:], 0.0)

            # ---------------- phase 0: table build ----------------
            NSLAB = 7
            SLAB = NPCP // NSLAB
            with (
                tc.tile_pool(name="p0x", bufs=2) as p0x,
                tc.tile_pool(name="p0ps", bufs=4, space="PSUM") as p0ps,
            ):
                for s in range(NSLAB):
                    xsl = p0x.tile([128, SLAB], fp32, tag="xsl")
                    nc.sync.dma_start(out=xsl[:],
                                      in_=xT[:, SLAB * s:SLAB * (s + 1)])
                    for tt in range(SLAB // 128):
                        t = s * (SLAB // 128) + tt
                        hps = p0ps.tile([128, ROW + HEADS], fp32,
                                        space="PSUM", tag="hps")
                        nc.tensor.matmul(out=hps[:],
                                         lhsT=xsl[:, 128 * tt:128 * (tt + 1)],
                                         rhs=w_sb[:], start=True, stop=True)
                        nc.vector.tensor_copy(out=tstag[:, t, :],
                                              in_=hps[:, 0:ROW])
                        nc.vector.tensor_copy(out=adst_nat[:, t, :],
                                              in_=hps[:, ROW:ROW + HEADS])
                nc.sync.dma_start(
                    out=tshard.ap().rearrange("(t p) d -> p t d", p=128),
                    in_=tstag[:])
                nc.gpsimd.collective_compute(
                    "AllGather", AL.bypass,
                    replica_groups=[list(range(NCORES))],
                    ins=[tshard.ap()],
                    outs=[agout.ap()],
                )

            # ---------------- phase 1: gather pipeline ----------------
            kcol = 0
            ccol = 0
            with (
                tc.tile_pool(name="p1g", bufs=2) as p1g,
                tc.tile_pool(name="p1i", bufs=3) as p1i,
                tc.tile_pool(name="p1h", bufs=2) as p1h,
                tc.tile_pool(name="p1f", bufs=2) as p1f,
                tc.tile_pool(name="p1s", bufs=2) as p1s,
            ):
                for group in _groups_of(D):
                    g_cols = int(sum(D[tau] for tau in group))
                    n_idx = 128 * g_cols
                    ix = p1i.tile([128, 8 * GROUP_COLS], i16, tag="ix")
                    nc.sync.dma_start(
                        out=ix[:, 0:n_idx // 16],
                        in_=gidx_in[:, kcol:kcol + n_idx // 16])
                    kcol += n_idx // 16
                    mk = p1i.tile([128, GROUP_COLS, 4], mybir.dt.int8, tag="mk")
                    nc.sync.dma_start(
                        out=mk[:, 0:g_cols, :],
                        in_=mask_in[:, 4 * ccol:4 * (ccol + g_cols)]
                        .rearrange("p (c q) -> p c q", q=4))
                    ccol += g_cols
                    gt = p1g.tile([128, GROUP_COLS, 4 * ROW], bf16, tag="gt")
                    nc.gpsimd.dma_gather(
                        out_ap=gt[:, 0:g_cols, :],
                        in_ap=tableQ,
                        idxs_ap=ix[:, 0:n_idx // 16],
                        num_idxs=n_idx, num_idxs_reg=n_idx, elem_size=4 * ROW,
                        elem_step=4 * ROW,
                        single_packet=False)
                    # 4-way select via predicated copies: hsel = gt[.., q(m)]
                    hsel = p1h.tile([128, GROUP_COLS, ROW], fp32, tag="hsel")
                    nc.vector.tensor_copy(out=hsel[:, 0:g_cols, :],
                                          in_=gt[:, 0:g_cols, 0:ROW])
                    for q in range(1, 4):
                        mq = bass.AP(
                            tensor=mk.tensor,
                            offset=mk[:, 0, q].offset,
                            ap=[mk.ap[0], [4, g_cols], [0, ROW]])
                        nc.vector.copy_predicated(
                            hsel[:, 0:g_cols, :], mq,
                            gt[:, 0:g_cols, q * ROW:(q + 1) * ROW])
                    for (tau0, nt, Dv) in _runs_of(D, group):
                        o = int(sum(D[tau] for tau in group[:group.index(tau0)]))
                        nd = nt * Dv
                        # s = a_src + a_dst ; a_src at lanes 16h+15
                        asrc_b = bass.AP(
                            tensor=hsel.tensor,
                            offset=hsel[:, o, 15].offset,
                            ap=[hsel.ap[0], [ROW, nd], [16, HEADS]])
                        adst_b = bass.AP(
                            tensor=adst_nat.tensor,
                            offset=adst_nat[:, tau0, :].offset,
                            ap=[adst_nat.ap[0], [HEADS, nt], [0, Dv],
                                [1, HEADS]])
                        sv = p1s.tile([128, RUN_COLS, HEADS], fp32, tag="sv")
                        nc.vector.tensor_tensor(out=sv[:, 0:nd, :],
                                                in0=asrc_b, in1=adst_b,
                                                op=AL.add)
                        ev = p1s.tile([128, RUN_COLS, HEADS], fp32, tag="ev")
                        nc.vector.tensor_scalar_mul(ev[:, 0:nd, :],
                                                    sv[:, 0:nd, :], NEG_SLOPE)
                        nc.vector.tensor_tensor(out=ev[:, 0:nd, :],
                                                in0=sv[:, 0:nd, :],
                                                in1=ev[:, 0:nd, :], op=AL.max)
                        wv = p1s.tile([128, RUN_COLS, HEADS], fp32, tag="wv")
                        nc.scalar.activation(out=wv[:, 0:nd, :],
                                             in_=ev[:, 0:nd, :], func=AF.Exp)
                        # numerator: prod = hsel * w (lane 16h+15 garbage)
                        w_b = bass.AP(tensor=wv.tensor, offset=wv.offset,
                                      ap=[wv.ap[0], [HEADS, nd], [1, HEADS],
                                          [0, 16]])
                        prod = p1f.tile([128, RUN_COLS, ROW], fp32, tag="hf")
                        nc.vector.tensor_tensor(out=prod[:, 0:nd, :],
                                                in0=hsel[:, o:o + nd, :],
                                                in1=w_b, op=AL.mult)
                        pt = bass.AP(tensor=prod.tensor, offset=prod.offset,
                                     ap=[prod.ap[0], [ROW * Dv, nt], [1, ROW],
                                         [ROW, Dv]])
                        nc.vector.tensor_reduce(
                            out=sstag[:, tau0:tau0 + nt, 0:ROW],
                            in_=pt, axis=mybir.AxisListType.X, op=AL.add)
                        wt = bass.AP(tensor=wv.tensor, offset=wv.offset,
                                     ap=[wv.ap[0], [HEADS * Dv, nt],
                                         [1, HEADS], [HEADS, Dv]])
                        nc.vector.tensor_reduce(
                            out=sstag[:, tau0:tau0 + nt, ROW:ROW + HEADS],
                            in_=wt, axis=mybir.AxisListType.X, op=AL.add)

            # ---------------- phase 2: combine ----------------
            with (
                tc.tile_pool(name="p2acc", bufs=1) as p2acc,
                tc.tile_pool(name="p2ps", bufs=2, space="PSUM") as p2ps,
                tc.tile_pool(name="p2t", bufs=4) as p2t,
            ):
                # self-loops: wl = exp(leaky(asrc_nat + adst_nat))
                tstagF = p2acc.tile([128, T, ROW], fp32)
                nc.vector.tensor_copy(out=tstagF[:], in_=tstag[:])
                asrc_nat = bass.AP(
                    tensor=tstagF.tensor,
                    offset=tstagF[:, 0, 15].offset,
                    ap=[tstagF.ap[0], [ROW, T], [16, HEADS]])
                ls = p2acc.tile([128, T, HEADS], fp32)
                wl = p2acc.tile([128, T, HEADS], fp32)
                nc.vector.tensor_tensor(out=ls[:], in0=asrc_nat,
                                        in1=adst_nat[:], op=AL.add)
                nc.vector.tensor_scalar_mul(wl[:], ls[:], NEG_SLOPE)
                nc.vector.tensor_tensor(out=wl[:], in0=ls[:], in1=wl[:],
                                        op=AL.max)
                nc.scalar.activation(out=wl[:], in_=wl[:], func=AF.Exp)
                # num += h_nat * wl ; den += wl
                wl_b = bass.AP(tensor=wl.tensor, offset=wl.offset,
                               ap=[wl.ap[0], [HEADS, T], [1, HEADS], [0, 16]])
                nc.vector.tensor_tensor(out=tstagF[:], in0=tstagF[:],
                                        in1=wl_b, op=AL.mult)
                nc.vector.tensor_tensor(out=sstag[:, :, 0:ROW],
                                        in0=sstag[:, :, 0:ROW], in1=tstagF[:],
                                        op=AL.add)
                nc.vector.tensor_tensor(out=sstag[:, :, ROW:ROW + HEADS],
                                        in0=sstag[:, :, ROW:ROW + HEADS],
                                        in1=wl[:], op=AL.add)
                # pad correction: each pad contributes exp(leaky(a_dst)) to den
                lsf = p2acc.tile([128, T, HEADS], fp32)
                mn4 = p2acc.tile([128, T, HEADS], fp32)
                nc.vector.tensor_scalar_mul(mn4[:], adst_nat[:], NEG_SLOPE)
                nc.vector.tensor_tensor(out=lsf[:], in0=adst_nat[:],
                                        in1=mn4[:], op=AL.max)
                nc.scalar.activation(out=lsf[:], in_=lsf[:], func=AF.Exp)
                p_b = bass.AP(tensor=p_sb.tensor, offset=p_sb.offset,
                              ap=[p_sb.ap[0], [1, T], [0, HEADS]])
                nc.vector.tensor_tensor(out=lsf[:], in0=lsf[:], in1=p_b,
                                        op=AL.mult)
                nc.vector.tensor_tensor(out=sstag[:, :, ROW:ROW + HEADS],
                                        in0=sstag[:, :, ROW:ROW + HEADS],
                                        in1=lsf[:], op=AL.subtract)
                nc.vector.tensor_scalar_max(sstag[:, :, ROW:ROW + HEADS],
                                            sstag[:, :, ROW:ROW + HEADS],
                                            1e-30)
                rec = p2acc.tile([128, T, HEADS], fp32)
                nc.vector.reciprocal(out=rec[:],
                                     in_=sstag[:, :, ROW:ROW + HEADS])
                y = p2acc.tile([128, T, ROW], fp32)
                rec_b = bass.AP(tensor=rec.tensor, offset=rec.offset,
                                ap=[rec.ap[0], [HEADS, T], [1, HEADS],
                                    [0, 16]])
                nc.vector.tensor_tensor(out=y[:], in0=sstag[:, :, 0:ROW],
                                        in1=rec_b, op=AL.mult)
                bias_b = bass.AP(tensor=bias_sb.tensor, offset=bias_sb.offset,
                                 ap=[bias_sb.ap[0], [0, T], [1, ROW]])
                nc.vector.tensor_tensor(out=y[:], in0=y[:], in1=bias_b,
                                        op=AL.add)
                # elu(y) = max(y,0) + exp(min(y,0)) - 1
                mn = p2acc.tile([128, T, ROW], fp32)
                nc.vector.tensor_scalar_min(mn[:], y[:], 0.0)
                nc.scalar.activation(out=mn[:], in_=mn[:], func=AF.Exp)
                nc.vector.tensor_scalar_max(y[:], y[:], 0.0)
                nc.vector.tensor_scalar_add(y[:], y[:], -1.0)
                nc.vector.tensor_tensor(out=y[:], in0=y[:], in1=mn[:],
                                        op=AL.add)
                zt_all = p2acc.tile([128, T, OUT_DIM], fp32)
                for t in range(T):
                    zps = p2ps.tile([ROW, 128], fp32, space="PSUM", tag="zps")
                    nc.tensor.transpose(out=zps[:], in_=y[:, t, :],
                                        identity=ident[:])
                    zT = p2t.tile([ROW, 128], fp32, tag="zT")
                    nc.vector.tensor_copy(out=zT[:], in_=zps[:])
                    ops_ = p2ps.tile([128, OUT_DIM], fp32, space="PSUM",
                                     tag="ops")
                    nc.tensor.matmul(out=ops_[:], lhsT=zT[:], rhs=linw_sb[:],
                                     start=True, stop=True)
                    nc.vector.tensor_copy(out=zt_all[:, t, :], in_=ops_[:])
                linb_b = bass.AP(tensor=linb_sb.tensor, offset=linb_sb.offset,
                                 ap=[linb_sb.ap[0], [0, T], [1, OUT_DIM]])
                nc.vector.tensor_tensor(out=zt_all[:], in0=zt_all[:],
                                        in1=linb_b, op=AL.add)
                mx = p2acc.tile([128, T, 1], fp32)
                nc.vector.tensor_reduce(out=mx[:], in_=zt_all[:],
                                        axis=mybir.AxisListType.X, op=AL.max)
                mx_b = bass.AP(tensor=mx.tensor, offset=mx.offset,
                               ap=[mx.ap[0], [1, T], [0, OUT_DIM]])
                nc.vector.tensor_tensor(out=zt_all[:], in0=zt_all[:],
                                        in1=mx_b, op=AL.subtract)
                qe = p2acc.tile([128, T, OUT_DIM], fp32)
                nc.scalar.activation(out=qe[:], in_=zt_all[:], func=AF.Exp)
                ssum = p2acc.tile([128, T, 1], fp32)
                nc.vector.tensor_reduce(out=ssum[:], in_=qe[:],
                                        axis=mybir.AxisListType.X, op=AL.add)
                nc.scalar.activation(out=ssum[:], in_=ssum[:], func=AF.Ln)
                ssum_b = bass.AP(tensor=ssum.tensor, offset=ssum.offset,
                                 ap=[ssum.ap[0], [1, T], [0, OUT_DIM]])
                nc.vector.tensor_tensor(out=zt_all[:], in0=zt_all[:],
                                        in1=ssum_b, op=AL.subtract)
                nc.sync.dma_start(out=out_t, in_=zt_all[:])

    nc.compile()
    return nc


_PROGRAM_CACHE = {}
LAST_EXEC_NS = None
LAST_TRACE = None


def kernel(**inputs):
    import os
    from concourse.bass_utils import run_bass_kernel_spmd

    x = np.asarray(inputs["x"], dtype=np.float32)
    ei = np.asarray(inputs["edge_index"])
    W = np.asarray(inputs["W"], dtype=np.float32)
    att_src = np.asarray(inputs["att_src"], dtype=np.float32)
    att_dst = np.asarray(inputs["att_dst"], dtype=np.float32)
    bias = np.asarray(inputs["bias"], dtype=np.float32)
    lin_w = np.asarray(inputs["lin_w"], dtype=np.float32)
    lin_b = np.asarray(inputs["lin_b"], dtype=np.float32)

    src = ei[0].astype(np.int64)
    dst = ei[1].astype(np.int64)

    D, per_core = _preprocess(src, dst)
    COLS = int(D.sum())

    key = tuple(int(v) for v in D)
    if key not in _PROGRAM_CACHE:
        _PROGRAM_CACHE[key] = _build_program(D, COLS)
    nc = _PROGRAM_CACHE[key]

    # global table row of node j: shard(core) * SH + rank within core
    rank_of_global = np.empty(N, np.int64)
    for c in range(NCORES):
        oi = per_core[c]["oi"][:NPC]
        rank_of_global[c * NPC:(c + 1) * NPC] = c * SH + oi

    # packed weights: cols [h0(15) ws0 | h1 ws1 | h2 ws2 | h3 ws3 | wd(4)]
    ws_arr = np.einsum('ihc,hc->ih', W.reshape(IN_DIM, HEADS, HID),
                       att_src).astype(np.float32)
    wd_arr = np.einsum('ihc,hc->ih', W.reshape(IN_DIM, HEADS, HID),
                       att_dst).astype(np.float32)
    Wr = W.reshape(IN_DIM, HEADS, HID)
    wpack = np.zeros((IN_DIM, ROW + HEADS), np.float32)
    for h in range(HEADS):
        wpack[:, 16 * h:16 * h + 15] = Wr[:, h, :]
        wpack[:, 16 * h + 15] = ws_arr[:, h]
    wpack[:, ROW:] = wd_arr

    bias_pack = np.zeros(ROW, np.float32)
    linw_pack = np.zeros((ROW, OUT_DIM), np.float32)
    br = bias.reshape(HEADS, HID)
    lr = lin_w.reshape(HEADS, HID, OUT_DIM)
    for h in range(HEADS):
        bias_pack[16 * h:16 * h + 15] = br[h]
        linw_pack[16 * h:16 * h + 15] = lr[h]
    bias_arr = np.tile(bias_pack.reshape(1, ROW), (128, 1)).astype(np.float32)
    linb_arr = np.tile(lin_b.reshape(1, OUT_DIM), (128, 1)).astype(np.float32)

    in_maps = []
    for c in range(NCORES):
        gidx, masks, Parr, COLS_c = _build_core_arrays(D, per_core[c],
                                                       rank_of_global)
        assert COLS_c == COLS
        # x in rank order: column j = x of rank j (global order[j])
        order = per_core[c]["order"]
        xs = np.zeros((NPCP, IN_DIM), np.float32)
        valid = order < NPC
        xs[valid] = x[c * NPC + order[valid]]
        im = {
            "xT": np.ascontiguousarray(xs.T),
            "w_in": wpack,
            "bias_in": bias_arr,
            "linw_in": linw_pack,
            "linb_in": linb_arr,
            "gidx_in": gidx,
            "mask_in": np.ascontiguousarray(masks.reshape(128, 4 * COLS)),
            "p_in": Parr,
        }
        in_maps.append(im)

    trace = os.environ.get("KERNEL_TRACE") == "1"
    res = run_bass_kernel_spmd(nc, in_maps, list(range(NCORES)), trace=trace)
    global LAST_EXEC_NS, LAST_TRACE
    LAST_EXEC_NS = res.exec_time_ns
    LAST_TRACE = res.instructions_and_trace[1] if res.instructions_and_trace else None

    out = np.empty((N, OUT_DIM), np.float32)
    for c in range(NCORES):
        buf = np.asarray(res.results[c]["out"])  # [128, T, OUT_DIM], rank r at (r%128, r//128)
        flat = buf.transpose(1, 0, 2).reshape(NPCP, OUT_DIM)
        order = per_core[c]["order"]
        valid = order < NPC
        out[c * NPC + order[valid]] = flat[valid]
    return out
